# revision 7
# baseline (speedup 1.0000x reference)
"""Trainium2 Bass kernel: poly_2-normalized attention (Newton row-solve).

Math per (b, h) slab:
  S  = Q @ K^T                       [L, L]  (raw, un-scaled)
  x  = S / sqrt(D)
  c0 = -max_k(x) - 1                 per row
  6x Newton:  u = -x - c ; ps = sum u^-2 ; psd = 2*sum u^-3
              c <- c - (ps - 1) / (psd + 1e-8)
  W  = u(c6)^-2
  Out = W @ V                        [L, D]

Sharding: 24 (b,h) slabs over 8 cores, 3 slabs/core, fully local.

This version splits each slab's 4 work units (4 q-chunks of 128 rows
each) between two Newton pipelines that run CONCURRENTLY on different
engines (units alternate D, A, D, A):

  A-units (ACT/scalar engine, 3 passes/chunk-iter):
      t = Ln(-x/8 + Bc);  exp(-2t) +accum -> ps;  exp(-3t) +accum -> psd
  D-units (DVE/vector engine, 2 passes/chunk-iter via two NEW custom
      DVE ops using the bitcast-reciprocal identity 1/x' = n*(1/g),
      n = bitcast(~x'), g = x'*n in [-4.5, -4]):
      SRSQ: rho2 = ((g*c1+c0)*n)^2, accum -> S2   (x' = S + 8c fused)
      SRCU: rho3 = rho2_in * rho,   accum -> S3
      ps = 64*S2, psd = -1024*S3 (folded into the stats chain)

Final weights for ALL chunks via SRSQ (W = rho2, bf16); the 64x score
fold rides the V copy (vsr = 64*V, bf16).  W^T via PE transposes
(bf16), output matmul V-stationary (bf16 x bf16), Out^T transposed
back on the PE.  matmul1 runs f32r (full fp32 bits, ~2.8x faster
column streaming).  PSUM evictions and W^T copies run on GpSimd.
"""

import numpy as np

B, L, H, D = 2, 2048, 12, 64
NCORES = 8
PAIRS = B * H           # 24 (b, h) slabs
SPC = PAIRS // NCORES   # 3 slabs per core
P = 128                 # SBUF partitions
NCH = L // P            # 16 q-chunks per slab
UNIT = 4                # q-chunks per work unit (quarter slab)
NUNITS = NCH // UNIT    # 4 units per slab
KB = 512                # matmul1 free-dim tile (one PSUM bank)
NEWTON_ITERS = 6
EPS = 1e-8
SCALE = float(-1.0 / np.sqrt(D))  # -0.125

# deg-1 minimax of 1/g on [-4.5, -4]:  p(g) = PC0 + PC1*g
PC0 = -0.4739293768005444
PC1 = -0.05607136778295589

# unit type per unit index within a slab: 1 = ACT path, 0 = DVE path
UNIT_PATTERN = (0, 1, 0, 1)

_CACHE = {}
_OPS = {}


def _register_ops():
    """Register the two fused Newton ops with the custom-DVE registry."""
    if _OPS:
        return _OPS["SRSQ"], _OPS["SRCU"]
    from operator import add
    import concourse.dve_ops as dvo
    from concourse.dve_spec import AluOp, Bin, Spec, Src0, Src1, Zero
    from concourse.dve_spec import C0, C1, C2, lower, sq
    from concourse.dve_uop import DveOpSpec

    _xp = Src0 + C0                       # x' = S + b'   (b' = 8c via s0)
    _n = Bin(AluOp.BITWISE_NOT, _xp, _xp)
    _rho = ((_xp * _n) * C1 + C2) * _n    # ~ 1/x'  (s1 = PC1, imm2 = PC0)

    def _rho_np(in0, s0, s1, imm2):
        xp = np.ascontiguousarray(
            in0.astype(np.float32) + np.asarray(s0, np.float32)
        )
        n = (~xp.view(np.int32)).view(np.float32)
        g = xp * n
        return (g * np.float32(s1) + np.float32(imm2)) * n

    def _ref_srsq(in0, in1, s0, s1, imm2):
        rho = _rho_np(in0, s0, s1, imm2)
        b = (rho * rho).astype(np.float32)
        return b, b.reshape(b.shape[0], -1).sum(axis=-1, keepdims=True)

    def _ref_srcu(in0, in1, s0, s1, imm2):
        rho = _rho_np(in0, s0, s1, imm2)
        b = (in1.astype(np.float32).reshape(rho.shape) * rho).astype(np.float32)
        return b, b.reshape(b.shape[0], -1).sum(axis=-1, keepdims=True)

    defs = [
        ("SHIFT_RECIP_SQ_REDUCE_ANT",
         Spec(body=sq(_rho), accum=add, accum_init=Zero, reference=_ref_srsq),
         False),
        ("SHIFT_RECIP_CUBE_REDUCE_ANT",
         Spec(body=_rho * Src1, accum=add, accum_init=Zero,
              reference=_ref_srcu),
         True),
    ]
    out = []
    for name, spec, rd1 in defs:
        if name not in dvo._SUB_OPCODE_FOR_NAME:
            opcode = dvo._CUSTOM_DVE_ROW_BASE + len(dvo.OPS)
            assert opcode < 0x20
            dvo._SUB_OPCODE_FOR_NAME[name] = opcode
            sha = {
                ver: DveOpSpec(name=name, opcode=opcode,
                               uops=lower(spec, ver=ver),
                               rd1_en=rd1).sha(ver)
                for ver in ("v3",)
            }
            op = dvo.DveOp(name, spec, subdim=False, uops_sha=sha)
            dvo.OPS.append(op)
            dvo.CUSTOM_DVE_SPECS[name] = spec
        else:
            op = next(o for o in dvo.OPS if o.name == name)
        out.append(op)
    _OPS["SRSQ"], _OPS["SRCU"] = out
    return out


def _pin_act_tables(bacc_mod, mybir):
    """Keep Ln/Exp servable only by natural_log_exp_and_others so the ATL
    chooser never reloads tables between passes."""
    import concourse.hw_specs as hw_specs
    AF = mybir.ActivationFunctionType
    pin = {AF.Ln, AF.Exp}
    orig = hw_specs.get_activation_tables

    def patched(arch):
        tabs = {k: set(v) for k, v in orig(arch).items()}
        for name, funcs in tabs.items():
            if name != "natural_log_exp_and_others":
                funcs -= pin
        return tabs

    bacc_mod.get_activation_tables = patched


def _build(spc=SPC):
    import concourse.bacc as bacc
    import concourse.tile as tile
    from concourse import mybir
    from concourse.masks import make_identity

    SRSQ, SRCU = _register_ops()
    _pin_act_tables(bacc, mybir)

    f32 = mybir.dt.float32
    f32r = mybir.dt.float32r
    bf16 = mybir.dt.bfloat16
    AX = mybir.AxisListType.X
    AF = mybir.ActivationFunctionType
    OP = mybir.AluOpType

    nc = bacc.Bacc(trn_type="TRN2", debug=False)
    q_d = nc.declare_dram_parameter("q", [spc, L, D], f32, isOutput=False)
    k_d = nc.declare_dram_parameter("k", [spc, L, D], f32, isOutput=False)
    v_d = nc.declare_dram_parameter("v", [spc, L, D], f32, isOutput=False)
    o_d = nc.declare_dram_parameter("o", [spc, L, D], f32, isOutput=True)

    with tile.TileContext(nc) as tc:
        with (
            tc.tile_pool(name="singles", bufs=1) as singles,
            tc.tile_pool(name="slabio", bufs=1) as slabio,
            tc.tile_pool(name="vpool", bufs=1) as vpool,
            tc.tile_pool(name="stage", bufs=1) as stage,
            tc.tile_pool(name="xpool", bufs=2) as xpool,
            tc.tile_pool(name="scr1", bufs=1) as scr1,
            tc.tile_pool(name="dscr", bufs=1) as dscr,
            tc.tile_pool(name="wev", bufs=2) as wev,
            tc.tile_pool(name="stats", bufs=2) as stats,
            tc.tile_pool(name="outb", bufs=2) as outb,
            tc.tile_pool(name="psx", bufs=2, space="PSUM") as psx,
            tc.tile_pool(name="pstr", bufs=2, space="PSUM") as pstr,
            tc.tile_pool(name="pso", bufs=2, space="PSUM") as pso,
        ):
            ident = singles.tile([P, P], f32)
            make_identity(nc, ident)
            identb = singles.tile([P, P], bf16)
            nc.vector.tensor_copy(out=identb, in_=ident)
            ident64 = singles.tile([64, 64], f32)
            make_identity(nc, ident64)

            for s in range(spc):
                # ---------- phase A: load + build Q^T, K^T (f32r), V ----------
                qt = slabio.tile([64, L], f32r, tag="qt")
                kt = slabio.tile([64, L], f32r, tag="kt")
                vsb = vpool.tile([P, NCH, D], f32, tag="vsb")
                nc.sync.dma_start(
                    out=vsb, in_=v_d[s].rearrange("(t p) d -> p t d", p=P)
                )
                # 64*V in bf16: folds the W = 64*rho^2 scale into matmul2
                vsr = vpool.tile([P, NCH, D], bf16, tag="vsr")
                nc.vector.tensor_scalar(
                    out=vsr, in0=vsb, scalar1=64.0, scalar2=None, op0=OP.mult,
                )
                for name, src, dst in (("q", q_d, qt), ("k", k_d, kt)):
                    sb = stage.tile([P, NCH, D], f32, tag="qkstage")
                    nc.sync.dma_start(
                        out=sb, in_=src[s].rearrange("(t p) d -> p t d", p=P)
                    )
                    for g in range(NCH // 4):  # groups of 4 transposes
                        ps_t = pstr.tile([64, 512], f32, tag="tr")
                        for j in range(4):
                            t = g * 4 + j
                            nc.tensor.transpose(
                                out=ps_t[:, j * P:(j + 1) * P],
                                in_=sb[:, t, :],
                                identity=ident,
                            )
                        nc.vector.tensor_copy(
                            out=dst[:, g * 512:(g + 1) * 512], in_=ps_t
                        )

                for un in range(NUNITS):
                    is_act = UNIT_PATTERN[un]
                    # ---------- phase B: matmul1 + evict + row max ----------
                    xsl = xpool.tile([P, UNIT, L], f32, tag="x")
                    mx = stats.tile([P, UNIT], f32, tag="mx")
                    bb = stats.tile([P, UNIT], f32, tag="bb")
                    for ci in range(UNIT):
                        qc = un * UNIT + ci
                        for g in range(L // KB):  # four 512-wide granules
                            ps_x = psx.tile([P, KB], f32, tag="x")
                            nc.tensor.matmul(
                                out=ps_x,
                                lhsT=qt[:, qc * P:(qc + 1) * P],
                                rhs=kt[:, g * KB:(g + 1) * KB],
                                start=True, stop=True,
                            )
                            nc.vector.tensor_copy(
                                out=xsl[:, ci, g * KB:(g + 1) * KB],
                                in_=ps_x,
                            )
                        nc.vector.reduce_max(
                            out=mx[:, ci:ci + 1], in_=xsl[:, ci, :], axis=AX
                        )
                        if is_act:
                            # Bc0 = -c0 = max(x)/8 + 1
                            nc.vector.tensor_scalar(
                                out=bb[:, ci:ci + 1], in0=mx[:, ci:ci + 1],
                                scalar1=-SCALE, scalar2=1.0,
                                op0=OP.mult, op1=OP.add,
                            )
                        else:
                            # b'0 = 8*c0 = -max(x) - 8  (raw-score shift)
                            nc.vector.tensor_scalar(
                                out=bb[:, ci:ci + 1], in0=mx[:, ci:ci + 1],
                                scalar1=-1.0, scalar2=-8.0,
                                op0=OP.mult, op1=OP.add,
                            )

                    # ---------- phase C: 6 Newton iterations ----------
                    for it in range(NEWTON_ITERS):
                        ps_t = stats.tile([P, UNIT], f32, tag="ps")
                        psd_t = stats.tile([P, UNIT], f32, tag="psd")
                        for ci in range(UNIT):
                            x_c = xsl[:, ci, :]
                            bb_c = bb[:, ci:ci + 1]
                            if is_act:
                                t_sc = scr1.tile([P, L], f32, tag="t_sc")
                                a_dump = scr1.tile([P, L], f32, tag="a_dump")
                                nc.scalar.activation(
                                    out=t_sc, in_=x_c, func=AF.Ln,
                                    bias=bb_c, scale=SCALE,
                                )
                                nc.scalar.activation(
                                    out=a_dump, in_=t_sc, func=AF.Exp,
                                    scale=-2.0, accum_out=ps_t[:, ci:ci + 1],
                                )
                                nc.scalar.activation(
                                    out=a_dump, in_=t_sc, func=AF.Exp,
                                    scale=-3.0, accum_out=psd_t[:, ci:ci + 1],
                                )
                            else:
                                rho2 = dscr.tile([P, L], f32, tag="rho2")
                                junk = dscr.tile([P, L], bf16, tag="junk")
                                nc.vector._custom_dve(
                                    SRSQ, out=rho2, in0=x_c,
                                    s0=bb_c, s1=PC1, imm2=PC0,
                                    accum_out=ps_t[:, ci:ci + 1],
                                )
                                nc.vector._custom_dve(
                                    SRCU, out=junk, in0=x_c, in1=rho2,
                                    s0=bb_c, s1=PC1, imm2=PC0,
                                    accum_out=psd_t[:, ci:ci + 1],
                                )
                        # stats update (tiny [P, UNIT] chain on DVE)
                        pden = stats.tile([P, UNIT], f32, tag="pden")
                        pr = stats.tile([P, UNIT], f32, tag="pr")
                        dcb = stats.tile([P, UNIT], f32, tag="dcb")
                        bb_new = stats.tile([P, UNIT], f32, tag="bb")
                        if is_act:
                            # Bc <- Bc + (ps-1)/(2*psd3 + eps)
                            nc.vector.tensor_scalar(
                                out=pden, in0=psd_t, scalar1=2.0, scalar2=EPS,
                                op0=OP.mult, op1=OP.add,
                            )
                            nc.vector.reciprocal(out=pr, in_=pden)
                            nc.vector.scalar_tensor_tensor(
                                out=dcb, in0=ps_t, scalar=-1.0, in1=pr,
                                op0=OP.add, op1=OP.mult,
                            )
                        else:
                            # b' <- b' + (S2 - 1/64)/(2*S3 - eps/512)
                            nc.vector.tensor_scalar(
                                out=pden, in0=psd_t, scalar1=2.0,
                                scalar2=-EPS / 512.0,
                                op0=OP.mult, op1=OP.add,
                            )
                            nc.vector.reciprocal(out=pr, in_=pden)
                            nc.vector.scalar_tensor_tensor(
                                out=dcb, in0=ps_t, scalar=-1.0 / 64.0, in1=pr,
                                op0=OP.add, op1=OP.mult,
                            )
                        nc.vector.tensor_add(out=bb_new, in0=bb, in1=dcb)
                        bb = bb_new

                    # ---------- phase W/E: weights, transpose, matmul2 -------
                    if is_act:
                        bp = stats.tile([P, UNIT], f32, tag="bp")
                        nc.vector.tensor_scalar(
                            out=bp, in0=bb, scalar1=-8.0, scalar2=None,
                            op0=OP.mult,
                        )
                    else:
                        bp = bb
                    jnk = stats.tile([P, UNIT], f32, tag="jnk")
                    wt_halves = [
                        wev.tile([P, NCH, 2 * P], bf16, tag="wt",
                                 name=f"wt{hi}")
                        for hi in range(2)
                    ]
                    for ci in range(UNIT):
                        x_c = xsl[:, ci, :]
                        w_sc = wev.tile([P, L], bf16, tag="w_sc")
                        nc.vector._custom_dve(
                            SRSQ, out=w_sc, in0=x_c,
                            s0=bp[:, ci:ci + 1], s1=PC1, imm2=PC0,
                            accum_out=jnk[:, ci:ci + 1],
                        )
                        wt_h = wt_halves[ci // 2]
                        qoff = (ci % 2) * P
                        for g in range(NCH // 4):
                            ps_t = pstr.tile([P, 512], bf16, tag="trb")
                            for j in range(4):
                                kcb = g * 4 + j
                                nc.tensor.transpose(
                                    out=ps_t[:, j * P:(j + 1) * P],
                                    in_=w_sc[:, kcb * P:(kcb + 1) * P],
                                    identity=identb,
                                )
                            nc.scalar.copy(
                                out=wt_h[:, g * 4:(g + 1) * 4,
                                         qoff:qoff + P],
                                in_=ps_t.rearrange("p (j q) -> p j q", j=4),
                            )
                    for hi in range(2):
                        # Out^T[d, q256] = sum_k (64V)[k, d]^T W^T[k, q256]
                        acc_t = pso.tile([64, 2 * P], f32, tag="ot")
                        for j in range(NCH):
                            nc.tensor.matmul(
                                out=acc_t, lhsT=vsr[:, j, :],
                                rhs=wt_halves[hi][:, j, :],
                                start=(j == 0), stop=(j == NCH - 1),
                            )
                        ot_sb = outb.tile([64, 2 * P], f32, tag="ot_sb")
                        nc.scalar.copy(out=ot_sb, in_=acc_t)
                        ps_o = pso.tile([P, P], f32, tag="ot")
                        for qi in range(2):
                            nc.tensor.transpose(
                                out=ps_o[:, qi * 64:(qi + 1) * 64],
                                in_=ot_sb[:, qi * P:(qi + 1) * P],
                                identity=ident64,
                            )
                        osb = outb.tile([P, P], f32, tag="osb")
                        nc.scalar.copy(out=osb, in_=ps_o)
                        q0 = (un * UNIT + hi * 2) * P
                        for qi in range(2):
                            nc.sync.dma_start(
                                out=o_d[s, q0 + qi * P:q0 + (qi + 1) * P, :],
                                in_=osb[:, qi * 64:(qi + 1) * 64],
                            )
    nc.compile()
    return nc


def get_nc(spc=SPC):
    if spc not in _CACHE:
        _CACHE[spc] = _build(spc)
    return _CACHE[spc]


def _shard(a):
    """[B, L, H, D] -> per-core [SPC, L, D] contiguous stacks."""
    a = np.ascontiguousarray(np.asarray(a, dtype=np.float32))
    per_core = []
    for i in range(NCORES):
        sl = [a[(i * SPC + j) // H, :, (i * SPC + j) % H, :]
              for j in range(SPC)]
        per_core.append(np.ascontiguousarray(np.stack(sl, axis=0)))
    return per_core


def kernel(query, key, value, _trace=False, _trace_kwargs=None):
    from concourse.bass_utils import run_bass_kernel_spmd

    nc = get_nc()
    qs, ks, vs = _shard(query), _shard(key), _shard(value)
    in_maps = [{"q": qs[i], "k": ks[i], "v": vs[i]} for i in range(NCORES)]
    res = run_bass_kernel_spmd(
        nc, in_maps, list(range(NCORES)), trace=_trace,
        **(_trace_kwargs or {}),
    )
    out = np.empty((B, L, H, D), dtype=np.float32)
    for i in range(NCORES):
        o = res.results[i]["o"]
        for j in range(SPC):
            p = i * SPC + j
            out[p // H, :, p % H, :] = o[j]
    if _trace:
        return out, res
    return out


# revision 13
# speedup vs baseline: 1.5659x; 1.5659x over previous
"""Trainium2 Bass kernel: poly_2-normalized attention (Newton row-solve).

Math per (b, h) slab:
  S  = Q @ K^T                       [L, L]  (raw, un-scaled)
  x  = S / sqrt(D)
  c0 = -max_k(x) - 1                 per row
  6x Newton:  u = -x - c ; ps = sum u^-2 ; psd = 2*sum u^-3
              c <- c - (ps - 1) / (psd + 1e-8)
  W  = u(c6)^-2
  Out = W @ V                        [L, D]

Sharding: 24 (b,h) slabs over 8 cores, 3 slabs/core, fully local.

This version splits each slab's 4 work units (4 q-chunks of 128 rows
each) between two Newton pipelines that run CONCURRENTLY on different
engines (units alternate D, A, D, A):

  A-units (ACT/scalar engine, 3 passes/chunk-iter):
      t = Ln(-x/8 + Bc);  exp(-2t) +accum -> ps;  exp(-3t) +accum -> psd
  D-units (DVE/vector engine, 2 passes/chunk-iter via two NEW custom
      DVE ops using the bitcast-reciprocal identity 1/x' = n*(1/g),
      n = bitcast(~x'), g = x'*n in [-4.5, -4]):
      SRSQ: rho2 = ((g*c1+c0)*n)^2, accum -> S2   (x' = S + 8c fused)
      SRCU: rho3 = rho2_in * rho,   accum -> S3
      ps = 64*S2, psd = -1024*S3 (folded into the stats chain)

Final weights for ALL chunks via SRSQ (W = rho2, bf16); the 64x score
fold rides the V copy (vsr = 64*V, bf16).  W^T via PE transposes
(bf16), output matmul V-stationary (bf16 x bf16), Out^T transposed
back on the PE.  matmul1 runs f32r (full fp32 bits, ~2.8x faster
column streaming).  PSUM evictions and W^T copies run on GpSimd.
"""

import numpy as np

B, L, H, D = 2, 2048, 12, 64
NCORES = 8
PAIRS = B * H           # 24 (b, h) slabs
SPC = PAIRS // NCORES   # 3 slabs per core
P = 128                 # SBUF partitions
NCH = L // P            # 16 q-chunks per slab
UNIT = 4                # q-chunks per work unit (quarter slab)
NUNITS = NCH // UNIT    # 4 units per slab
KB = 512                # matmul1 free-dim tile (one PSUM bank)
NEWTON_ITERS = 6
EPS = 1e-8
SCALE = float(-1.0 / np.sqrt(D))  # -0.125

# deg-1 minimax of 1/g on [-4.5, -4]:  p(g) = PC0 + PC1*g
PC0 = -0.4739293768005444
PC1 = -0.05607136778295589

# unit type per unit index within a slab: 1 = ACT path, 0 = DVE path
UNIT_PATTERN = (0, 1, 0, 1)

_CACHE = {}
_OPS = {}


def _register_ops():
    """Register the two fused Newton ops with the custom-DVE registry."""
    if _OPS:
        return _OPS["SRSQ"], _OPS["SRCU"]
    from operator import add
    import concourse.dve_ops as dvo
    from concourse.dve_spec import AluOp, Bin, Spec, Src0, Src1, Zero
    from concourse.dve_spec import C0, C1, C2, lower, sq
    from concourse.dve_uop import DveOpSpec

    _xp = Src0 + C0                       # x' = S + b'   (b' = 8c via s0)
    _n = Bin(AluOp.BITWISE_NOT, _xp, _xp)
    _rho = ((_xp * _n) * C1 + C2) * _n    # ~ 1/x'  (s1 = PC1, imm2 = PC0)

    def _rho_np(in0, s0, s1, imm2):
        xp = np.ascontiguousarray(
            in0.astype(np.float32) + np.asarray(s0, np.float32)
        )
        n = (~xp.view(np.int32)).view(np.float32)
        g = xp * n
        return (g * np.float32(s1) + np.float32(imm2)) * n

    def _ref_srsq(in0, in1, s0, s1, imm2):
        rho = _rho_np(in0, s0, s1, imm2)
        b = (rho * rho).astype(np.float32)
        return b, b.reshape(b.shape[0], -1).sum(axis=-1, keepdims=True)

    def _ref_srcu(in0, in1, s0, s1, imm2):
        rho = _rho_np(in0, s0, s1, imm2)
        b = (in1.astype(np.float32).reshape(rho.shape) * rho).astype(np.float32)
        return b, b.reshape(b.shape[0], -1).sum(axis=-1, keepdims=True)

    defs = [
        ("SHIFT_RECIP_SQ_REDUCE_ANT",
         Spec(body=sq(_rho), accum=add, accum_init=Zero, reference=_ref_srsq),
         False),
        ("SHIFT_RECIP_CUBE_REDUCE_ANT",
         Spec(body=_rho * Src1, accum=add, accum_init=Zero,
              reference=_ref_srcu),
         True),
    ]
    out = []
    for name, spec, rd1 in defs:
        if name not in dvo._SUB_OPCODE_FOR_NAME:
            opcode = dvo._CUSTOM_DVE_ROW_BASE + len(dvo.OPS)
            assert opcode < 0x20
            dvo._SUB_OPCODE_FOR_NAME[name] = opcode
            sha = {
                ver: DveOpSpec(name=name, opcode=opcode,
                               uops=lower(spec, ver=ver),
                               rd1_en=rd1).sha(ver)
                for ver in ("v3",)
            }
            op = dvo.DveOp(name, spec, subdim=False, uops_sha=sha)
            dvo.OPS.append(op)
            dvo.CUSTOM_DVE_SPECS[name] = spec
        else:
            op = next(o for o in dvo.OPS if o.name == name)
        out.append(op)
    _OPS["SRSQ"], _OPS["SRCU"] = out
    return out


def _pin_act_tables(bacc_mod, mybir):
    """Keep Ln/Exp servable only by natural_log_exp_and_others so the ATL
    chooser never reloads tables between passes."""
    import concourse.hw_specs as hw_specs
    AF = mybir.ActivationFunctionType
    pin = {AF.Ln, AF.Exp}
    orig = hw_specs.get_activation_tables

    def patched(arch):
        tabs = {k: set(v) for k, v in orig(arch).items()}
        for name, funcs in tabs.items():
            if name != "natural_log_exp_and_others":
                funcs -= pin
        return tabs

    bacc_mod.get_activation_tables = patched


def _build(spc=SPC):
    import concourse.bacc as bacc
    import concourse.tile as tile
    from concourse import mybir
    from concourse.masks import make_identity

    SRSQ, SRCU = _register_ops()
    _pin_act_tables(bacc, mybir)

    f32 = mybir.dt.float32
    f32r = mybir.dt.float32r
    bf16 = mybir.dt.bfloat16
    AX = mybir.AxisListType.X
    AF = mybir.ActivationFunctionType
    OP = mybir.AluOpType

    nc = bacc.Bacc(trn_type="TRN2", debug=False)
    q_d = nc.declare_dram_parameter("q", [spc, L, D], f32, isOutput=False)
    k_d = nc.declare_dram_parameter("k", [spc, L, D], f32, isOutput=False)
    v_d = nc.declare_dram_parameter("v", [spc, L, D], f32, isOutput=False)
    o_d = nc.declare_dram_parameter("o", [spc, L, D], f32, isOutput=True)

    with tile.TileContext(nc) as tc:
        with (
            tc.tile_pool(name="singles", bufs=1) as singles,
            tc.tile_pool(name="slabio", bufs=1) as slabio,
            tc.tile_pool(name="vpool", bufs=1) as vpool,
            tc.tile_pool(name="stage", bufs=1) as stage,
            tc.tile_pool(name="xpool", bufs=3) as xpool,
            tc.tile_pool(name="scr1", bufs=1) as scr1,
            tc.tile_pool(name="dscr", bufs=1) as dscr,
            tc.tile_pool(name="wev", bufs=2) as wev,
            tc.tile_pool(name="stats", bufs=12) as stats,
            tc.tile_pool(name="outb", bufs=2) as outb,
            tc.tile_pool(name="psx", bufs=3, space="PSUM") as psx,
            tc.tile_pool(name="pstr", bufs=1, space="PSUM") as pstr,
            tc.tile_pool(name="pstrb", bufs=2, space="PSUM") as pstrb,
            tc.tile_pool(name="pso", bufs=2, space="PSUM") as pso,
        ):
            ident = singles.tile([P, P], f32)
            make_identity(nc, ident)
            identb = singles.tile([P, P], bf16)
            nc.vector.tensor_copy(out=identb, in_=ident)
            ident64 = singles.tile([64, 64], f32)
            make_identity(nc, ident64)

            for s in range(spc):
                # ---------- phase A: load + build Q^T, K^T (f32r), V ----------
                qt = slabio.tile([64, L], f32r, tag="qt")
                kt = slabio.tile([64, L], f32r, tag="kt")
                vsb = vpool.tile([P, NCH, D], f32, tag="vsb")
                nc.sync.dma_start(
                    out=vsb, in_=v_d[s].rearrange("(t p) d -> p t d", p=P)
                )
                # 64*V in bf16: folds the W = 64*rho^2 scale into matmul2
                vsr = vpool.tile([P, NCH, D], bf16, tag="vsr")
                nc.vector.tensor_scalar(
                    out=vsr, in0=vsb, scalar1=64.0, scalar2=None, op0=OP.mult,
                )
                for name, src, dst in (("q", q_d, qt), ("k", k_d, kt)):
                    sb = stage.tile([P, NCH, D], f32, tag="qkstage")
                    nc.sync.dma_start(
                        out=sb, in_=src[s].rearrange("(t p) d -> p t d", p=P)
                    )
                    for g in range(NCH // 4):  # groups of 4 transposes
                        ps_t = pstr.tile([64, 512], f32, tag="tr")
                        for j in range(4):
                            t = g * 4 + j
                            nc.tensor.transpose(
                                out=ps_t[:, j * P:(j + 1) * P],
                                in_=sb[:, t, :],
                                identity=ident,
                            )
                        nc.vector.tensor_copy(
                            out=dst[:, g * 512:(g + 1) * 512], in_=ps_t
                        )

                for pair in range(NUNITS // 2):
                    units = (2 * pair, 2 * pair + 1)  # (D-unit, A-unit)
                    xsl_u, mx_u, bb_u = {}, {}, {}
                    # ------ phase B: matmul1 + DMA evict + row max ------
                    for un in units:
                        is_act = UNIT_PATTERN[un]
                        sfx = "A" if is_act else "D"
                        xsl = xpool.tile([P, UNIT, L], f32, tag="x")
                        mx = stats.tile([P, UNIT], f32, tag="mx" + sfx)
                        bb = stats.tile([P, UNIT], f32, tag="bb" + sfx)
                        for ci in range(UNIT):
                            qc = un * UNIT + ci
                            for g in range(L // KB):
                                ps_x = psx.tile([P, KB], f32, tag="x")
                                nc.tensor.matmul(
                                    out=ps_x,
                                    lhsT=qt[:, qc * P:(qc + 1) * P],
                                    rhs=kt[:, g * KB:(g + 1) * KB],
                                    start=True, stop=True,
                                )
                                if is_act:
                                    nc.vector.tensor_copy(
                                        out=xsl[:, ci, g * KB:(g + 1) * KB],
                                        in_=ps_x,
                                    )
                                else:
                                    nc.scalar.copy(
                                        out=xsl[:, ci, g * KB:(g + 1) * KB],
                                        in_=ps_x,
                                    )
                            nc.vector.reduce_max(
                                out=mx[:, ci:ci + 1], in_=xsl[:, ci, :],
                                axis=AX,
                            )
                            if is_act:
                                # Bc0 = -c0 = max(x)/8 + 1
                                nc.vector.tensor_scalar(
                                    out=bb[:, ci:ci + 1],
                                    in0=mx[:, ci:ci + 1],
                                    scalar1=-SCALE, scalar2=1.0,
                                    op0=OP.mult, op1=OP.add,
                                )
                            else:
                                # b'0 = 8*c0 = -max(x) - 8 (raw-score shift)
                                nc.vector.tensor_scalar(
                                    out=bb[:, ci:ci + 1],
                                    in0=mx[:, ci:ci + 1],
                                    scalar1=-1.0, scalar2=-8.0,
                                    op0=OP.mult, op1=OP.add,
                                )
                        xsl_u[un], mx_u[un], bb_u[un] = xsl, mx, bb

                    # ------ phase C: 6 Newton iterations, D||A ------
                    for it in range(NEWTON_ITERS):
                        for un in units:
                            is_act = UNIT_PATTERN[un]
                            sfx = "A" if is_act else "D"
                            xsl, bb = xsl_u[un], bb_u[un]
                            ps_t = stats.tile([P, UNIT], f32, tag="ps" + sfx)
                            psd_t = stats.tile([P, UNIT], f32,
                                               tag="psd" + sfx)
                            for ci in range(UNIT):
                                x_c = xsl[:, ci, :]
                                bb_c = bb[:, ci:ci + 1]
                                if is_act:
                                    t_sc = scr1.tile([P, L], f32, tag="t_sc")
                                    a_dump = scr1.tile([P, L], f32,
                                                       tag="a_dump")
                                    nc.scalar.activation(
                                        out=t_sc, in_=x_c, func=AF.Ln,
                                        bias=bb_c, scale=SCALE,
                                    )
                                    nc.scalar.activation(
                                        out=a_dump, in_=t_sc, func=AF.Exp,
                                        scale=-2.0,
                                        accum_out=ps_t[:, ci:ci + 1],
                                    )
                                    nc.scalar.activation(
                                        out=a_dump, in_=t_sc, func=AF.Exp,
                                        scale=-3.0,
                                        accum_out=psd_t[:, ci:ci + 1],
                                    )
                                else:
                                    rho2 = dscr.tile([P, L], f32, tag="rho2")
                                    junk = dscr.tile([P, L], bf16,
                                                     tag="junk")
                                    nc.vector._custom_dve(
                                        SRSQ, out=rho2, in0=x_c,
                                        s0=bb_c, s1=PC1, imm2=PC0,
                                        accum_out=ps_t[:, ci:ci + 1],
                                    )
                                    nc.vector._custom_dve(
                                        SRCU, out=junk, in0=x_c, in1=rho2,
                                        s0=bb_c, s1=PC1, imm2=PC0,
                                        accum_out=psd_t[:, ci:ci + 1],
                                    )
                            # stats update (tiny [P, UNIT] chain on DVE)
                            pden = stats.tile([P, UNIT], f32,
                                              tag="pden" + sfx)
                            pr = stats.tile([P, UNIT], f32, tag="pr" + sfx)
                            dcb = stats.tile([P, UNIT], f32, tag="dcb" + sfx)
                            bb_new = stats.tile([P, UNIT], f32,
                                                tag="bb" + sfx)
                            if is_act:
                                # Bc <- Bc + (ps-1)/(2*psd3 + eps)
                                nc.vector.tensor_scalar(
                                    out=pden, in0=psd_t, scalar1=2.0,
                                    scalar2=EPS, op0=OP.mult, op1=OP.add,
                                )
                                nc.vector.reciprocal(out=pr, in_=pden)
                                nc.vector.scalar_tensor_tensor(
                                    out=dcb, in0=ps_t, scalar=-1.0, in1=pr,
                                    op0=OP.add, op1=OP.mult,
                                )
                            else:
                                # b' <- b' + (S2 - 1/64)/(2*S3 - eps/512)
                                nc.vector.tensor_scalar(
                                    out=pden, in0=psd_t, scalar1=2.0,
                                    scalar2=-EPS / 512.0,
                                    op0=OP.mult, op1=OP.add,
                                )
                                nc.vector.reciprocal(out=pr, in_=pden)
                                nc.vector.scalar_tensor_tensor(
                                    out=dcb, in0=ps_t, scalar=-1.0 / 64.0,
                                    in1=pr, op0=OP.add, op1=OP.mult,
                                )
                            nc.vector.tensor_add(out=bb_new, in0=bb,
                                                 in1=dcb)
                            bb_u[un] = bb_new

                    # ------ phase W/E: weights, transpose, matmul2 ------
                    for un in units:
                        is_act = UNIT_PATTERN[un]
                        sfx = "A" if is_act else "D"
                        xsl, bb = xsl_u[un], bb_u[un]
                        if is_act:
                            bp = stats.tile([P, UNIT], f32, tag="bp")
                            nc.vector.tensor_scalar(
                                out=bp, in0=bb, scalar1=-8.0, scalar2=None,
                                op0=OP.mult,
                            )
                        else:
                            bp = bb
                        jnk = stats.tile([P, UNIT], f32, tag="jnk" + sfx)
                        wt_halves = [
                            wev.tile([P, NCH, 2 * P], bf16, tag="wt",
                                     name=f"wt{hi}")
                            for hi in range(2)
                        ]
                        for ci in range(UNIT):
                            x_c = xsl[:, ci, :]
                            w_sc = wev.tile([P, L], bf16, tag="w_sc")
                            nc.vector._custom_dve(
                                SRSQ, out=w_sc, in0=x_c,
                                s0=bp[:, ci:ci + 1], s1=PC1, imm2=PC0,
                                accum_out=jnk[:, ci:ci + 1],
                            )
                            wt_h = wt_halves[ci // 2]
                            qoff = (ci % 2) * P
                            for g in range(NCH // 4):
                                ps_t = pstrb.tile([P, 512], bf16, tag="trb")
                                for j in range(4):
                                    kcb = g * 4 + j
                                    nc.tensor.transpose(
                                        out=ps_t[:, j * P:(j + 1) * P],
                                        in_=w_sc[:, kcb * P:(kcb + 1) * P],
                                        identity=identb,
                                    )
                                wt_dst = wt_h[:, g * 4:(g + 1) * 4,
                                              qoff:qoff + P]
                                wt_src = ps_t.rearrange("p (j q) -> p j q",
                                                        j=4)
                                if is_act:
                                    nc.vector.tensor_copy(out=wt_dst,
                                                          in_=wt_src)
                                else:
                                    nc.scalar.copy(out=wt_dst, in_=wt_src)
                        for hi in range(2):
                            # Out^T[d, q256] = sum_k (64V)^T W^T[k, q256]
                            acc_t = pso.tile([64, 2 * P], f32, tag="ot")
                            for j in range(NCH):
                                nc.tensor.matmul(
                                    out=acc_t, lhsT=vsr[:, j, :],
                                    rhs=wt_halves[hi][:, j, :],
                                    start=(j == 0), stop=(j == NCH - 1),
                                )
                            ot_sb = outb.tile([64, 2 * P], f32, tag="ot_sb")
                            nc.scalar.copy(out=ot_sb, in_=acc_t)
                            ps_o = pso.tile([P, P], f32, tag="ot")
                            for qi in range(2):
                                nc.tensor.transpose(
                                    out=ps_o[:, qi * 64:(qi + 1) * 64],
                                    in_=ot_sb[:, qi * P:(qi + 1) * P],
                                    identity=ident64,
                                )
                            osb = outb.tile([P, P], f32, tag="osb")
                            nc.scalar.copy(out=osb, in_=ps_o)
                            q0 = (un * UNIT + hi * 2) * P
                            for qi in range(2):
                                nc.sync.dma_start(
                                    out=o_d[s, q0 + qi * P:q0 + (qi + 1) * P,
                                            :],
                                    in_=osb[:, qi * 64:(qi + 1) * 64],
                                )
    nc.compile()
    return nc


def get_nc(spc=SPC):
    if spc not in _CACHE:
        _CACHE[spc] = _build(spc)
    return _CACHE[spc]


def _shard(a):
    """[B, L, H, D] -> per-core [SPC, L, D] contiguous stacks."""
    a = np.ascontiguousarray(np.asarray(a, dtype=np.float32))
    per_core = []
    for i in range(NCORES):
        sl = [a[(i * SPC + j) // H, :, (i * SPC + j) % H, :]
              for j in range(SPC)]
        per_core.append(np.ascontiguousarray(np.stack(sl, axis=0)))
    return per_core


def kernel(query, key, value, _trace=False, _trace_kwargs=None):
    from concourse.bass_utils import run_bass_kernel_spmd

    nc = get_nc()
    qs, ks, vs = _shard(query), _shard(key), _shard(value)
    in_maps = [{"q": qs[i], "k": ks[i], "v": vs[i]} for i in range(NCORES)]
    res = run_bass_kernel_spmd(
        nc, in_maps, list(range(NCORES)), trace=_trace,
        **(_trace_kwargs or {}),
    )
    out = np.empty((B, L, H, D), dtype=np.float32)
    for i in range(NCORES):
        o = res.results[i]["o"]
        for j in range(SPC):
            p = i * SPC + j
            out[p // H, :, p % H, :] = o[j]
    if _trace:
        return out, res
    return out
